# revision 3
# baseline (speedup 1.0000x reference)
"""Trainium2 Bass kernel v2 for nn_CombinedHiddenEncoder.

Same algebraic restructure as v1 (S^3 applied to a fused input projection,
head folded into V @ [Wm|Wv] plus host-computed rank-1 corrections), with:

  * bf16 X tables: halves AllGather wire and enables 1-cycle/row matmuls.
  * Self-loop edges removed from the gather lists; their contribution is a
    per-tile DVE add of the previous strip (X_{r+1} = dinv^2*(sum + X_r)).
  * Source-block split replaces the int16 parity split: the per-round table
    is AllGathered as two blocks, each block table has < 32768 rows so
    gather indices fit int16 directly with contiguous rows.
  * Runtime chunk schedule: chunks per (dst tile, src block) are
    max over cores of ceil(edge count / 128) instead of a global max,
    cutting one-hot matmul + gather-slot padding.
  * Overlap mode: AllGather of block b of round r+1 is issued as soon as
    the first B0T (resp. all 49) dst tiles of round r are finalized, hiding
    the collective behind the next round's gather stream.

Gathers are transaction-bound (~8ns/descriptor), so slot-count reduction is
what matters; bf16 mainly buys matmul rate and collective bytes.

build_program(repeat=K) unrolls the whole pipeline K times in one NEFF —
used to measure device time below the ~2.2ms per-call dispatch floor.
"""

import numpy as np
import ml_dtypes

import concourse.bass as bass
import concourse.mybir as mybir
import concourse.tile as tile
from concourse import bacc
from concourse.bass_utils import run_bass_kernel_spmd
from concourse.masks import make_identity

F32 = mybir.dt.float32
BF16 = mybir.dt.bfloat16
I16 = mybir.dt.int16
NPBF16 = ml_dtypes.bfloat16

# ---- problem constants ----
N, E = 50000, 600000
FD, CD, HD, LD = 256, 128, 128, 64
CORES = 8
SHARD = N // CORES            # 6250
TILES = (SHARD + 127) // 128  # 49
R = TILES * 128               # 6272
B0T = 18                      # src tiles in block 0 (B1T=31: 31744-row table < 32768)
B1T = TILES - B0T
B0ROWS, B1ROWS = B0T * 128, B1T * 128
TB0, TB1 = CORES * B0ROWS, CORES * B1ROWS
GROUP = 7
NGROUPS = TILES // GROUP      # 7

_prog_cache: dict = {}


# --------------------------------------------------------------------------
# Bass program builder
# --------------------------------------------------------------------------
def build_program(sched, overlap=True, variant="full", repeat=1):
    """sched: tuple of (TILES, 2) chunk counts (dst tile, src block)."""
    sched = np.asarray(sched, np.int64).reshape(TILES, 2)
    do_gather = variant not in ("nogather",)
    do_mm = variant not in ("nomm",)
    do_cc = variant not in ("nocc",)
    n_rounds = int(variant[1:]) if variant.startswith("r") else 3

    nc = bacc.Bacc(None, target_bir_lowering=False, num_swdge_queues=4)

    call_chunks = {}
    for b in (0, 1):
        for g in range(NGROUPS):
            call_chunks[(b, g)] = int(
                sum(sched[g * GROUP + i, b] for i in range(GROUP)))
    total_chunks = int(sched.sum())
    total_icols = total_chunks * 8        # 128 idx per chunk / 16 partitions

    # ---- I/O ----
    xfT = nc.dram_tensor("xfT", [FD, R], BF16, kind="ExternalInput")
    xcT = nc.dram_tensor("xcT", [CD, R], BF16, kind="ExternalInput")
    noise_in = nc.dram_tensor("noise_in", [R, LD], F32, kind="ExternalInput")
    aw = nc.dram_tensor("aw", [FD, HD], BF16, kind="ExternalInput")
    bw = nc.dram_tensor("bw", [CD, HD], BF16, kind="ExternalInput")
    wm = nc.dram_tensor("wm", [HD, LD], BF16, kind="ExternalInput")
    wv = nc.dram_tensor("wv", [HD, LD], BF16, kind="ExternalInput")
    cm = nc.dram_tensor("cm", [4, LD], BF16, kind="ExternalInput")
    cv = nc.dram_tensor("cv", [4, LD], BF16, kind="ExternalInput")
    dinv_p = nc.dram_tensor("dinv_p", [128, TILES], F32, kind="ExternalInput")
    dinv2_p = nc.dram_tensor("dinv2_p", [128, TILES], F32, kind="ExternalInput")
    srows = nc.dram_tensor("srows", [4, R], BF16, kind="ExternalInput")
    idx_all = nc.dram_tensor("idx_all", [128, total_icols], I16,
                             kind="ExternalInput")
    dstloc_all = nc.dram_tensor("dstloc_all", [128, total_chunks], F32,
                                kind="ExternalInput")

    z_out = nc.dram_tensor("z_out", [R, LD], F32, kind="ExternalOutput")
    mean_out = nc.dram_tensor("mean_out", [R, LD], F32, kind="ExternalOutput")
    logvar_out = nc.dram_tensor("logvar_out", [R, LD], F32,
                                kind="ExternalOutput")

    # ---- internal DRAM: per-round, per-block bounce + gathered tables ----
    brows = (B0ROWS, B1ROWS)
    trows = (TB0, TB1)
    bounce = [[nc.dram_tensor(f"xb{r}_{b}", [brows[b], HD], BF16)
               for b in (0, 1)] for r in range(3)]
    tabs = [[nc.dram_tensor(f"tab{r}_{b}", [trows[b], HD], BF16,
                            addr_space="Shared")
             for b in (0, 1)] for r in range(3)]
    rg = [list(range(CORES))]

    # idx column offsets per call, in pack order (b, g)
    icol_off = {}
    off = 0
    for b in (0, 1):
        for g in range(NGROUPS):
            icol_off[(b, g)] = off
            off += call_chunks[(b, g)] * 8
    # global chunk column index per (b, tile)
    chunk_off = {}
    off = 0
    for b in (0, 1):
        for g in range(NGROUPS):
            for i in range(GROUP):
                t = g * GROUP + i
                chunk_off[(b, t)] = off
                off += int(sched[t, b])

    def emit_ag(rnd, b):
        if do_cc:
            nc.gpsimd.collective_compute(
                "AllGather", mybir.AluOpType.bypass, replica_groups=rg,
                ins=[bounce[rnd][b].ap()], outs=[tabs[rnd][b].ap()])

    def emit_bounce(rnd, b, strip):
        c0 = 0 if b == 0 else B0ROWS
        rows = brows[b]
        nc.sync.dma_start(
            out=bounce[rnd][b].ap().rearrange("(t p) h -> p t h", p=128),
            in_=strip[:, c0:c0 + rows].rearrange("p (t h) -> p t h", h=HD))

    with tile.TileContext(nc) as tc:
        with tc.tile_pool(name="const", bufs=1) as cpool:
            ident = cpool.tile([128, 128], BF16)
            make_identity(nc, ident[:])
            colidx = cpool.tile([128, 128], BF16)
            nc.gpsimd.iota(colidx[:], pattern=[[1, 128]], base=0,
                           channel_multiplier=0,
                           allow_small_or_imprecise_dtypes=True)
            a0_s = cpool.tile([128, HD], BF16)
            a1_s = cpool.tile([128, HD], BF16)
            b_s = cpool.tile([128, HD], BF16)
            nc.sync.dma_start(out=a0_s[:], in_=aw[0:128, :])
            nc.sync.dma_start(out=a1_s[:], in_=aw[128:256, :])
            nc.sync.dma_start(out=b_s[:], in_=bw[:, :])
            wm_s = cpool.tile([128, LD], BF16)
            wv_s = cpool.tile([128, LD], BF16)
            nc.sync.dma_start(out=wm_s[:], in_=wm[:, :])
            nc.sync.dma_start(out=wv_s[:], in_=wv[:, :])
            cm_s = cpool.tile([4, LD], BF16)
            cv_s = cpool.tile([4, LD], BF16)
            nc.sync.dma_start(out=cm_s[:4, :], in_=cm[:, :])
            nc.sync.dma_start(out=cv_s[:4, :], in_=cv[:, :])
            dinv_s = cpool.tile([128, TILES], F32)
            dinv2_s = cpool.tile([128, TILES], F32)
            nc.sync.dma_start(out=dinv_s[:], in_=dinv_p[:, :])
            nc.sync.dma_start(out=dinv2_s[:], in_=dinv2_p[:, :])
            idx_s = cpool.tile([128, total_icols], I16)
            nc.sync.dma_start(out=idx_s[:], in_=idx_all[:, :])
            dloc_s = cpool.tile([128, total_chunks], F32)
            nc.sync.dma_start(out=dloc_s[:], in_=dstloc_all[:, :])
            strips = [cpool.tile([128, R], BF16, name=f"strip{i}")
                      for i in range(2)]

            with tc.tile_pool(name="psum", bufs=4, space="PSUM") as mmpool, \
                 tc.tile_pool(name="hpsum", bufs=4, space="PSUM") as hpool, \
                 tc.tile_pool(name="s0", bufs=1) as s0pool, \
                 tc.tile_pool(name="gpool", bufs=4) as gpool, \
                 tc.tile_pool(name="qpool", bufs=6) as qpool, \
                 tc.tile_pool(name="hd", bufs=3) as hdpool:
                for rep in range(repeat):
                    sx = f"_{rep}" if repeat > 1 else ""

                    # ------------ stage 0: X0 = T (Xf A + Xc B) -------------
                    xf0_s = s0pool.tile([128, R], BF16, name=f"xf0{sx}",
                                        tag="xf0")
                    xf1_s = s0pool.tile([128, R], BF16, name=f"xf1{sx}",
                                        tag="xf1")
                    xc_s = s0pool.tile([128, R], BF16, name=f"xc{sx}",
                                       tag="xc")
                    nc.sync.dma_start(out=xf0_s[:], in_=xfT[0:128, :])
                    nc.sync.dma_start(out=xf1_s[:], in_=xfT[128:256, :])
                    nc.sync.dma_start(out=xc_s[:], in_=xcT[:, :])
                    for t in range(TILES):
                        cs = slice(t * 128, (t + 1) * 128)
                        ps = mmpool.tile([128, HD], F32, name=f"s0ps{t}{sx}",
                                         tag="mm")
                        nc.tensor.matmul(ps[:], xf0_s[:, cs], a0_s[:],
                                         start=True, stop=False)
                        nc.tensor.matmul(ps[:], xf1_s[:, cs], a1_s[:],
                                         start=False, stop=False)
                        nc.tensor.matmul(ps[:], xc_s[:, cs], b_s[:],
                                         start=False, stop=True)
                        nc.vector.tensor_scalar(
                            out=strips[0][:, cs], in0=ps[:],
                            scalar1=dinv_s[:, t:t + 1], scalar2=None,
                            op0=mybir.AluOpType.mult)
                        if overlap and t == B0T - 1:
                            emit_bounce(0, 0, strips[0])
                            emit_ag(0, 0)
                    if overlap:
                        emit_bounce(0, 1, strips[0])
                        emit_ag(0, 1)
                    else:
                        emit_bounce(0, 0, strips[0])
                        emit_bounce(0, 1, strips[0])
                        emit_ag(0, 0)
                        emit_ag(0, 1)

                    # ------------ sparse rounds -----------------------------
                    for rnd in range(n_rounds):
                        last = rnd == n_rounds - 1
                        xprev = strips[rnd % 2]
                        xnext = strips[(rnd + 1) % 2]
                        sc_s = dinv_s if last else dinv2_s
                        for b in (0, 1):
                            for g in range(NGROUPS):
                                cpc = call_chunks[(b, g)]
                                gt = gpool.tile([128, cpc, 128], BF16,
                                                name=f"gt{rnd}_{b}_{g}{sx}",
                                                tag="gath")
                                if do_gather:
                                    ic0 = icol_off[(b, g)]
                                    nc.gpsimd.dma_gather(
                                        out_ap=gt[:],
                                        in_ap=tabs[rnd][b].ap(),
                                        idxs_ap=idx_s[:, ic0:ic0 + cpc * 8],
                                        num_idxs=cpc * 128,
                                        num_idxs_reg=cpc * 128,
                                        elem_size=HD,
                                        elem_step=HD,
                                        single_packet=False,
                                        queue_num=(b * NGROUPS + g) % 4)
                                else:
                                    nc.vector.tensor_copy(out=gt[:, 0, :],
                                                          in_=colidx[:])
                                base = chunk_off[(b, g * GROUP)]
                                for i in range(GROUP):
                                    t = g * GROUP + i
                                    nch = int(sched[t, b])
                                    goff = base - chunk_off[(b, g * GROUP)]
                                    ps = mmpool.tile(
                                        [128, HD], F32,
                                        name=f"ps{rnd}_{b}_{t}{sx}", tag="mm")
                                    nmm = nch if do_mm else 1
                                    for c in range(nmm):
                                        col = base + c
                                        q = qpool.tile(
                                            [128, 128], BF16,
                                            name=f"q{rnd}_{b}_{t}_{c}{sx}",
                                            tag="q")
                                        nc.vector.tensor_scalar(
                                            out=q[:], in0=colidx[:],
                                            scalar1=dloc_s[:, col:col + 1],
                                            scalar2=None,
                                            op0=mybir.AluOpType.is_equal)
                                        nc.tensor.matmul(
                                            ps[:], q[:],
                                            gt[:, goff + c, :],
                                            start=(c == 0),
                                            stop=(c == nmm - 1))
                                    base += nch
                                    cs = slice(t * 128, (t + 1) * 128)
                                    if b == 0:
                                        nc.vector.tensor_copy(
                                            out=xnext[:, cs], in_=ps[:])
                                        continue
                                    # finalize: scale*(P0 + P1 + Xprev)
                                    pa = hdpool.tile([128, HD], BF16,
                                                     name=f"pa{rnd}_{t}{sx}",
                                                     tag="pa")
                                    nc.vector.tensor_scalar(
                                        out=pa[:], in0=ps[:],
                                        scalar1=sc_s[:, t:t + 1], scalar2=None,
                                        op0=mybir.AluOpType.mult)
                                    pb = hdpool.tile([128, HD], BF16,
                                                     name=f"pb{rnd}_{t}{sx}",
                                                     tag="pb")
                                    nc.vector.tensor_tensor(
                                        out=pb[:], in0=xnext[:, cs],
                                        in1=xprev[:, cs],
                                        op=mybir.AluOpType.add)
                                    nc.vector.tensor_scalar(
                                        out=pb[:], in0=pb[:],
                                        scalar1=sc_s[:, t:t + 1], scalar2=None,
                                        op0=mybir.AluOpType.mult)
                                    nc.vector.tensor_tensor(
                                        out=xnext[:, cs], in0=pa[:], in1=pb[:],
                                        op=mybir.AluOpType.add)
                                    if not last:
                                        if overlap and t == B0T - 1:
                                            emit_bounce(rnd + 1, 0, xnext)
                                            emit_ag(rnd + 1, 0)
                                        elif overlap and t == TILES - 1:
                                            emit_bounce(rnd + 1, 1, xnext)
                                            emit_ag(rnd + 1, 1)
                                        continue
                                    # ---- head (last round, block 1) ----
                                    pst = mmpool.tile([128, HD], BF16,
                                                      name=f"pst{t}{sx}",
                                                      tag="mm")
                                    nc.tensor.transpose(pst[:], xnext[:, cs],
                                                        ident[:])
                                    vT = hdpool.tile([128, HD], BF16,
                                                     name=f"vT{t}{sx}",
                                                     tag="vT")
                                    nc.vector.tensor_copy(out=vT[:], in_=pst[:])
                                    sr = hdpool.tile([4, 128], BF16,
                                                     name=f"sr{t}{sx}",
                                                     tag="sr")
                                    nc.sync.dma_start(out=sr[:4, :],
                                                      in_=srows[:, cs])
                                    nz = hdpool.tile([128, LD], F32,
                                                     name=f"nz{t}{sx}",
                                                     tag="nz")
                                    nc.sync.dma_start(out=nz[:],
                                                      in_=noise_in[cs, :])
                                    mps = hpool.tile([128, LD], F32,
                                                     name=f"mps{t}{sx}",
                                                     tag="hp")
                                    nc.tensor.matmul(mps[:], vT[:], wm_s[:],
                                                     start=True, stop=False)
                                    nc.tensor.matmul(mps[:], sr[:3, :],
                                                     cm_s[:3, :],
                                                     start=False, stop=True)
                                    lps = hpool.tile([128, LD], F32,
                                                     name=f"lps{t}{sx}",
                                                     tag="hp")
                                    nc.tensor.matmul(lps[:], vT[:], wv_s[:],
                                                     start=True, stop=False)
                                    nc.tensor.matmul(lps[:], sr[:3, :],
                                                     cv_s[:3, :],
                                                     start=False, stop=True)
                                    mn = hdpool.tile([128, LD], F32,
                                                     name=f"mn{t}{sx}",
                                                     tag="mn")
                                    lv = hdpool.tile([128, LD], F32,
                                                     name=f"lv{t}{sx}",
                                                     tag="lv")
                                    ex = hdpool.tile([128, LD], F32,
                                                     name=f"ex{t}{sx}",
                                                     tag="ex")
                                    zt = hdpool.tile([128, LD], F32,
                                                     name=f"zt{t}{sx}",
                                                     tag="zt")
                                    nc.vector.tensor_copy(out=mn[:], in_=mps[:])
                                    nc.vector.tensor_copy(out=lv[:], in_=lps[:])
                                    nc.scalar.activation(
                                        out=ex[:], in_=lps[:],
                                        func=mybir.ActivationFunctionType.Exp,
                                        scale=0.5)
                                    nc.vector.tensor_tensor(
                                        out=zt[:], in0=nz[:], in1=ex[:],
                                        op=mybir.AluOpType.mult)
                                    nc.vector.tensor_tensor(
                                        out=zt[:], in0=zt[:], in1=mn[:],
                                        op=mybir.AluOpType.add)
                                    nc.sync.dma_start(out=z_out[cs, :],
                                                      in_=zt[:])
                                    nc.sync.dma_start(out=mean_out[cs, :],
                                                      in_=mn[:])
                                    nc.sync.dma_start(out=logvar_out[cs, :],
                                                      in_=lv[:])
                        if not last and not overlap:
                            emit_bounce(rnd + 1, 0, xnext)
                            emit_bounce(rnd + 1, 1, xnext)
                            emit_ag(rnd + 1, 0)
                            emit_ag(rnd + 1, 1)
    nc.finalize()
    return nc


# --------------------------------------------------------------------------
# Host-side preprocessing
# --------------------------------------------------------------------------
def preprocess(feature, condition, edge_index, noise,
               W1, b1, W2, b2, W3, b3, Wm, bm, Wv, bv):
    feature = np.asarray(feature, np.float32)
    condition = np.asarray(condition, np.float32)
    noise = np.asarray(noise, np.float32)
    ei = np.asarray(edge_index).astype(np.int64)
    W1 = np.asarray(W1, np.float32); b1 = np.asarray(b1, np.float32)
    W2 = np.asarray(W2, np.float32); b2 = np.asarray(b2, np.float32)
    W3 = np.asarray(W3, np.float32); b3 = np.asarray(b3, np.float32)
    Wm = np.asarray(Wm, np.float32); bm = np.asarray(bm, np.float32)
    Wv = np.asarray(Wv, np.float32); bv = np.asarray(bv, np.float32)

    src, dst = ei[0], ei[1]
    loop = np.arange(N, dtype=np.int64)
    src_f = np.concatenate([src, loop])
    dst_f = np.concatenate([dst, loop])
    deg = np.bincount(dst_f, minlength=N).astype(np.float64)
    dinv = 1.0 / np.sqrt(deg)
    w = dinv[src_f] * dinv[dst_f]
    s1 = np.bincount(dst_f, weights=w, minlength=N)
    s2 = np.bincount(dst_f, weights=w * s1[src_f], minlength=N)
    dinv32 = dinv.astype(np.float32)

    W3a, W3b = W3[:HD], W3[HD:]
    A_w = W1 @ W3a
    B_w = W2 @ W3b
    c1 = b1 @ W3a + b2 @ W3b
    Cm = np.zeros((4, LD), np.float32)
    Cm[:3] = np.stack([c1 @ Wm, b3 @ Wm, bm])
    Cv = np.zeros((4, LD), np.float32)
    Cv[:3] = np.stack([c1 @ Wv, b3 @ Wv, bv])

    # ---- edge bookkeeping (no self loops) ----
    score = src // SHARD
    slocal = src - score * SHARD
    blk = (slocal >= B0ROWS).astype(np.int64)
    trow = np.where(blk == 0, score * B0ROWS + slocal,
                    score * B1ROWS + (slocal - B0ROWS))
    core = dst // SHARD
    d_loc = dst - core * SHARD
    tl = d_loc // 128
    dstloc = d_loc % 128

    gid = (core * TILES + tl) * 2 + blk
    ngroups_tot = CORES * TILES * 2
    counts = np.bincount(gid, minlength=ngroups_tot).reshape(CORES, TILES, 2)
    sched = np.maximum(1, -(-counts.max(axis=0) // 128))     # [TILES, 2]
    total_chunks = int(sched.sum())

    # slot base per (tile, blk): chunk order is (b, g, t, c)
    chunk_base = np.zeros((TILES, 2), np.int64)
    off = 0
    for b in (0, 1):
        for g in range(NGROUPS):
            for i in range(GROUP):
                t = g * GROUP + i
                chunk_base[t, b] = off
                off += int(sched[t, b])

    order = np.argsort(gid, kind="stable")
    gs = gid[order]
    cnt_flat = counts.reshape(-1)
    within = np.arange(len(gs)) - np.repeat(
        np.concatenate([[0], np.cumsum(cnt_flat)[:-1]]), cnt_flat)
    t_o = (gs // 2) % TILES
    b_o = gs % 2
    slot = (chunk_base[t_o, b_o] * 128 + within).astype(np.int64)

    idx_slots = np.zeros((CORES, total_chunks * 128), np.int16)
    dl_slots = np.full((CORES, total_chunks * 128), -1.0, np.float32)
    core_o = gs // (TILES * 2)
    idx_slots[core_o, slot] = trow[order].astype(np.int16)
    dl_slots[core_o, slot] = dstloc[order].astype(np.float32)

    call_sizes = []
    for b in (0, 1):
        for g in range(NGROUPS):
            call_sizes.append(int(sum(sched[g * GROUP + i, b]
                                      for i in range(GROUP))) * 128)
    call_ends = np.cumsum(call_sizes)

    in_maps = []
    for k in range(CORES):
        rows = slice(k * SHARD, (k + 1) * SHARD)
        xfT = np.zeros((FD, R), NPBF16)
        xfT[:, :SHARD] = feature[rows].T.astype(NPBF16)
        xcT = np.zeros((CD, R), NPBF16)
        xcT[:, :SHARD] = condition[rows].T.astype(NPBF16)
        nz = np.zeros((R, LD), np.float32)
        nz[:SHARD] = noise[rows]
        dv = np.zeros((TILES, 128), np.float32)
        dv.reshape(-1)[:SHARD] = dinv32[rows]
        sr = np.zeros((4, R), np.float32)
        di = dinv[rows.start:rows.stop]
        sr[0, :SHARD] = (s2[rows] / di).astype(np.float32)
        sr[1, :SHARD] = (s1[rows] / di).astype(np.float32)
        sr[2, :SHARD] = (1.0 / di).astype(np.float32)

        ics = []
        for cs_end, csz in zip(call_ends, call_sizes):
            blkv = idx_slots[k, cs_end - csz:cs_end]
            ics.append(blkv.reshape(csz // 16, 16))
        ic = np.concatenate(ics, axis=0).T           # [16, total/16]
        idx_arr = np.tile(np.ascontiguousarray(ic), (8, 1))

        dl_arr = np.ascontiguousarray(
            dl_slots[k].reshape(total_chunks, 128).T)

        in_maps.append({
            "xfT": xfT, "xcT": xcT, "noise_in": nz,
            "aw": A_w.astype(NPBF16), "bw": B_w.astype(NPBF16),
            "wm": Wm.astype(NPBF16), "wv": Wv.astype(NPBF16),
            "cm": Cm.astype(NPBF16), "cv": Cv.astype(NPBF16),
            "dinv_p": np.ascontiguousarray(dv.T),
            "dinv2_p": np.ascontiguousarray((dv ** 2).T),
            "srows": sr.astype(NPBF16),
            "idx_all": np.ascontiguousarray(idx_arr),
            "dstloc_all": dl_arr,
        })
    return tuple(map(tuple, sched)), in_maps


def kernel(feature, condition, edge_index, noise,
           W1, b1, W2, b2, W3, b3, Wm, bm, Wv, bv,
           _trace=False, _overlap=True, _variant="full"):
    sched, in_maps = preprocess(feature, condition, edge_index, noise,
                                W1, b1, W2, b2, W3, b3, Wm, bm, Wv, bv)
    key = (sched, _overlap, _variant)
    if key not in _prog_cache:
        _prog_cache[key] = build_program(sched, _overlap, _variant)
    nc = _prog_cache[key]
    res = run_bass_kernel_spmd(nc, in_maps, list(range(CORES)), trace=_trace)
    z = np.concatenate([res.results[k]["z_out"][:SHARD] for k in range(CORES)])
    mean = np.concatenate(
        [res.results[k]["mean_out"][:SHARD] for k in range(CORES)])
    logvar = np.concatenate(
        [res.results[k]["logvar_out"][:SHARD] for k in range(CORES)])
    if _trace:
        kernel._last_exec_time_ns = res.exec_time_ns
        kernel._last_results = res
    return (z, mean, logvar)

